# revision 15
# baseline (speedup 1.0000x reference)
"""MoE (4 MLP experts + 4 FasterKAN experts, top-2 routing) on 8 Trainium2
NeuronCores.

Strategy: the gate (tiny: [2048,1024]@[1024,8]) is computed on host exactly
mirroring the reference ops.  Tokens are dispatched sparsely to their selected
experts; each expert's token list is chopped into <=512-token chunks and the 8
chunks (one per core) are processed by a single SPMD Bass kernel: every core
runs one KAN expert chunk (rswaf basis -> spline matmul -> rswaf -> spline
matmul) and one MLP expert chunk (fc1 -> gelu -> fc2), bf16 matmuls with fp32
accumulation.  Host scatter-adds the weighted expert outputs.

Self-contained: hardcodes T=2048, H=1024, F=4096, E=8, G=8, top_k=2.
"""

import math

import numpy as np
import ml_dtypes

import concourse.bass as bass
import concourse.mybir as mybir
from concourse.tile import TileContext
from concourse.bass_utils import run_bass_kernel_spmd

# ---------------------------------------------------------------- constants
P = 128
T, H, F, E = 2048, 1024, 4096, 8
G = 8  # rswaf grids
TOP_K = 2
NM = E // 2  # experts per type
F2 = F // 2  # KAN hidden width

HT = H // P  # 8 h-tiles
FT1 = F2 // P  # 16 f-tiles (KAN layer 1 out)
KT1 = HT * G  # 64 k-tiles (KAN layer 1 contraction: H*G)
KT2 = FT1 * G  # 128 k-tiles (KAN layer 2 contraction: F2*G)
FTM = F // P  # 32 f-tiles (MLP hidden)

BF16 = ml_dtypes.bfloat16

_GRID = None  # lazy: match jnp.linspace bitwise


def _grid():
    global _GRID
    if _GRID is None:
        import jax.numpy as jnp

        _GRID = np.asarray(jnp.linspace(-1.2, 0.2, G).astype(jnp.float32))
    return _GRID


# ------------------------------------------------- BIR fixup for this walrus
def _split_sync_waits(nc, max_waits=1):
    """This walrus build only allows one sync-wait command per instruction;
    move extra waits onto preceding same-engine NoOps."""
    for f in nc.m.functions:
        for bb in f.blocks:
            insts = bb.instructions
            new = []
            changed = False
            for inst in insts:
                si = inst.sync_info
                if si is not None and si.on_wait and len(si.on_wait) > max_waits:
                    waits = list(si.on_wait)
                    extra = waits[:-max_waits]
                    for s in range(0, len(extra), max_waits):
                        chunk = extra[s : s + max_waits]
                        new.append(
                            mybir.InstNoOp(
                                name=nc.get_next_instruction_name(),
                                engine=inst.engine,
                                sync_info=mybir.SyncInfo(
                                    on_wait=list(chunk), on_update=[]
                                ),
                                bass_nofuse=True,
                            )
                        )
                    while len(si.on_wait) > max_waits:
                        si.on_wait.pop(0)
                    changed = True
                new.append(inst)
            if changed:
                bb.instructions = new


# ------------------------------------------------------------ device kernel
def _build_nc(CK, CM):
    """Build the per-core SPMD program.

    Each core computes, for its private weights/tokens:
      KAN:  b1 = rswaf(xk) [H*G]; k1 = b1 @ w1 -> [F2]; b2 = rswaf(k1);
            yk = b2 @ w2 -> [H]
      MLP:  h = gelu(xm @ fw1 + fb1); ym = h @ fw2 + fb2
    Token-transposed layout throughout: tokens live on the free dim.
    """
    f32 = mybir.dt.float32
    bf16 = mybir.dt.bfloat16
    TANH = mybir.ActivationFunctionType.Tanh
    GELU = mybir.ActivationFunctionType.Gelu
    IDENT = mybir.ActivationFunctionType.Identity
    MULT = mybir.AluOpType.mult
    ADD = mybir.AluOpType.add

    nc = bass.Bass()
    xkT = nc.dram_tensor("xkT", [H, CK], f32, kind="ExternalInput")
    xmT = nc.dram_tensor("xmT", [H, CM], f32, kind="ExternalInput")
    # KAN L1 weights: [ft, i, kt, j]; kt = g*HT + ht; contraction index (ht*P+i, g)
    w1 = nc.dram_tensor("w1", [FT1, P, KT1, P], bf16, kind="ExternalInput")
    # KAN L2 weights: [kt2, i, ht2, j]; kt2 = g*FT1 + ft
    w2 = nc.dram_tensor("w2", [KT2, P, HT, P], bf16, kind="ExternalInput")
    # MLP weights: fw1 [ft, i, ht, j], fw2 [ht, i, ft, j]
    fw1 = nc.dram_tensor("fw1", [FTM, P, HT, P], bf16, kind="ExternalInput")
    fw2 = nc.dram_tensor("fw2", [HT, P, FTM, P], bf16, kind="ExternalInput")
    fb1 = nc.dram_tensor("fb1", [P, FTM], f32, kind="ExternalInput")
    fb2 = nc.dram_tensor("fb2", [P, HT], f32, kind="ExternalInput")
    # per-grid tanh biases, replicated over partitions: gb[:, g] = -0.5*grid[g]
    gb = nc.dram_tensor("gb", [P, G], f32, kind="ExternalInput")
    ykT = nc.dram_tensor("ykT", [H, CK], f32, kind="ExternalOutput")
    ymT = nc.dram_tensor("ymT", [H, CM], f32, kind="ExternalOutput")

    xk_r = xkT.rearrange("(o p) c -> p o c", p=P)
    xm_r = xmT.rearrange("(o p) c -> p o c", p=P)
    yk_r = ykT.rearrange("(o p) c -> p o c", p=P)
    ym_r = ymT.rearrange("(o p) c -> p o c", p=P)

    with TileContext(nc) as tc:
        with (
            tc.tile_pool(name="persist", bufs=1) as persist,
            tc.tile_pool(name="scratch", bufs=4) as scratch,
            tc.tile_pool(name="wstream", bufs=2) as wstream,
            tc.tile_pool(name="w2stream", bufs=3) as w2stream,
            tc.tile_pool(name="outp", bufs=3) as outp,
        ):
            # ---- persistent SBUF tensors
            xk_sb = persist.tile([P, HT, CK], f32, tag="xk")
            xm_sb = persist.tile([P, HT, CM], f32, tag="xm")
            xmb_sb = persist.tile([P, HT, CM], bf16, tag="xmb")
            b1_sb = persist.tile([P, KT1, CK], bf16, tag="b1")
            k1_sb = persist.tile([P, FT1, CK], f32, tag="k1")
            h1_sb = persist.tile([P, FTM, CM], bf16, tag="h1")
            fb1_sb = persist.tile([P, FTM], f32, tag="fb1")
            fb2_sb = persist.tile([P, HT], f32, tag="fb2")
            gb_sb = persist.tile([P, G], f32, tag="gb")

            nc.sync.dma_start(xk_sb[:], xk_r)
            nc.sync.dma_start(xm_sb[:], xm_r)
            nc.sync.dma_start(fb1_sb[:], fb1[:])
            nc.sync.dma_start(fb2_sb[:], fb2[:])
            nc.sync.dma_start(gb_sb[:], gb[:])
            nc.vector.tensor_copy(xmb_sb[:], xm_sb[:])

            def basis_tile(src_ap, g, out_ap):
                # out = 1 - tanh(0.5*src - 0.5*grid[g])^2, bf16
                t = scratch.tile([P, CK], f32, tag="tanh")
                nc.scalar.activation(
                    t[:], src_ap, TANH, bias=gb_sb[:, g : g + 1], scale=0.5
                )
                sq = scratch.tile([P, CK], f32, tag="sq")
                nc.vector.tensor_tensor(sq[:], t[:], t[:], MULT)
                nc.vector.tensor_scalar(out_ap, sq[:], -1.0, 1.0, MULT, ADD)

            # emission: MLP L1 (with b1 production interleaved), MLP L2, K1, K2
            with (
                tc.tile_pool(name="ps_m1", bufs=3, space="PSUM") as ps_m1,
                tc.tile_pool(name="ps_m2", bufs=2, space="PSUM") as ps_m2,
                tc.tile_pool(name="ps_k1", bufs=3, space="PSUM") as ps_k1,
            ):
                # ---------------- MLP layer 1 + b1 basis production
                for ft in range(FTM):
                    fw1_sb = wstream.tile([P, HT, P], bf16, tag="fw1")
                    nc.sync.dma_start(fw1_sb[:], fw1[ft])
                    ps = ps_m1.tile([P, CM], f32, tag="m1")
                    for ht in range(HT):
                        nc.tensor.matmul(
                            ps[:],
                            fw1_sb[:, ht],
                            xmb_sb[:, ht],
                            start=(ht == 0),
                            stop=(ht == HT - 1),
                        )
                    nc.scalar.activation(
                        h1_sb[:, ft], ps[:], GELU, bias=fb1_sb[:, ft : ft + 1], scale=1.0
                    )
                    # interleave two b1 tiles per ft so ACT/DVE fill PE shadow
                    for kt in (2 * ft, 2 * ft + 1):
                        g, ht = divmod(kt, HT)
                        basis_tile(xk_sb[:, ht], g, b1_sb[:, kt])

                # ---------------- MLP layer 2
                for ht in range(HT):
                    fw2_sb = wstream.tile([P, FTM, P], bf16, tag="fw2")
                    nc.sync.dma_start(fw2_sb[:], fw2[ht])
                    ps = ps_m2.tile([P, CM], f32, tag="m2")
                    for ft in range(FTM):
                        nc.tensor.matmul(
                            ps[:],
                            fw2_sb[:, ft],
                            h1_sb[:, ft],
                            start=(ft == 0),
                            stop=(ft == FTM - 1),
                        )
                    out_sb = outp.tile([P, CM], f32, tag="ym")
                    nc.scalar.activation(
                        out_sb[:], ps[:], IDENT, bias=fb2_sb[:, ht : ht + 1], scale=1.0
                    )
                    nc.sync.dma_start(ym_r[:, ht], out_sb[:])

                # ---------------- KAN layer 1
                for ft in range(FT1):
                    w1_sb = wstream.tile([P, KT1, P], bf16, tag="w1")
                    nc.sync.dma_start(w1_sb[:], w1[ft])
                    ps = ps_k1.tile([P, CK], f32, tag="k1ps")
                    for kt in range(KT1):
                        nc.tensor.matmul(
                            ps[:],
                            w1_sb[:, kt],
                            b1_sb[:, kt],
                            start=(kt == 0),
                            stop=(kt == KT1 - 1),
                        )
                    nc.scalar.copy(k1_sb[:, ft], ps[:])

            # ---------------- KAN layer 2 (8 concurrent psum accumulators)
            with tc.tile_pool(name="ps_k2", bufs=1, space="PSUM") as ps_k2:
                accs = [
                    ps_k2.tile([P, CK], f32, tag=f"acc{h}", name=f"acc{h}")
                    for h in range(HT)
                ]
                for kt2 in range(KT2):
                    g, ft = divmod(kt2, FT1)
                    b2t = scratch.tile([P, CK], bf16, tag="b2")
                    basis_tile(k1_sb[:, ft], g, b2t[:])
                    w2_sb = w2stream.tile([P, HT, P], bf16, tag="w2")
                    nc.sync.dma_start(w2_sb[:], w2[kt2])
                    for ht2 in range(HT):
                        nc.tensor.matmul(
                            accs[ht2][:],
                            w2_sb[:, ht2],
                            b2t[:],
                            start=(kt2 == 0),
                            stop=(kt2 == KT2 - 1),
                        )
                for ht2 in range(HT):
                    out_sb = outp.tile([P, CK], f32, tag="yk")
                    nc.scalar.copy(out_sb[:], accs[ht2][:])
                    nc.sync.dma_start(yk_r[:, ht2], out_sb[:])

    _split_sync_waits(nc, max_waits=1)
    return nc


_NC_CACHE = {}
_LAST_IN_MAPS = None  # debug/timing hook for test.py


def _get_nc(CK, CM):
    key = (CK, CM)
    if key not in _NC_CACHE:
        _NC_CACHE[key] = _build_nc(CK, CM)
    return _NC_CACHE[key]


# ------------------------------------------------------------- host helpers
def _routing(hidden_states, gate_w):
    """Mirror the reference router bit-exactly (same ops, same order)."""
    import jax
    import jax.numpy as jnp

    logits = hidden_states @ gate_w.T
    probs = jax.nn.softmax(jnp.asarray(logits).astype(jnp.float32), axis=-1)
    rw, sel = jax.lax.top_k(probs, TOP_K)
    rw = rw / rw.sum(axis=-1, keepdims=True)
    return np.asarray(rw).astype(np.float32), np.asarray(sel)


def _chunk_capacity(counts, max_chunks):
    for C in range(32, 512 + 32, 32):
        if sum(-(-c // C) for c in counts if c) <= max_chunks:
            return C
    return 512  # multi-pass handles overflow


def _make_chunks(tok_lists, wgt_lists, C):
    """[(expert, token_index_array, weight_array), ...], each <= C tokens."""
    chunks = []
    for e, (toks, wgts) in enumerate(zip(tok_lists, wgt_lists)):
        for s in range(0, len(toks), C):
            chunks.append((e, toks[s : s + C], wgts[s : s + C]))
    return chunks


def _prep_kan_weights(sp1_w, sp2_w, e):
    # w1[ft, i, kt=g*HT+ht, j] = sp1_w[e][ft*P+j, (ht*P+i)*G+g]
    a = np.ascontiguousarray(
        sp1_w[e].reshape(FT1, P, HT, P, G).transpose(0, 3, 4, 2, 1)
    )  # [ft, i, g, ht, j]
    w1 = a.reshape(FT1, P, KT1, P).astype(BF16)
    # w2[kt2=g*FT1+ft, i, ht2, j] = sp2_w[e][ht2*P+j, (ft*P+i)*G+g]
    b = np.ascontiguousarray(
        sp2_w[e].reshape(HT, P, FT1, P, G).transpose(4, 2, 3, 0, 1)
    )  # [g, ft, i, ht2, j]
    w2 = b.reshape(KT2, P, HT, P).astype(BF16)
    return w1, w2


def _prep_mlp_weights(fc1_w, fc2_w, e):
    fw1 = np.ascontiguousarray(
        fc1_w[e].reshape(FTM, P, HT, P).transpose(0, 3, 2, 1)
    ).astype(BF16)
    fw2 = np.ascontiguousarray(
        fc2_w[e].reshape(HT, P, FTM, P).transpose(0, 3, 2, 1)
    ).astype(BF16)
    return fw1, fw2


def _gather_tokens_T(hidden_states, toks, C):
    """[len(toks), H] tokens -> zero-padded transposed [H, C] fp32."""
    out = np.zeros((H, C), np.float32)
    if len(toks):
        out[:, : len(toks)] = hidden_states[toks].T
    return out


# ------------------------------------------------------------------- kernel
def kernel(hidden_states, gate_w, fc1_w, fc1_b, fc2_w, fc2_b, sp1_w, sp2_w):
    hidden_states = np.asarray(hidden_states, np.float32)
    gate_w = np.asarray(gate_w, np.float32)

    rw, sel = _routing(hidden_states, gate_w)

    # per-expert token lists + combine weights
    tok = [[] for _ in range(E)]
    wgt = [[] for _ in range(E)]
    for k in range(TOP_K):
        for t_idx, e in enumerate(sel[:, k]):
            tok[e].append(t_idx)
            wgt[e].append(rw[t_idx, k])
    tok = [np.asarray(t, np.int64) for t in tok]
    wgt = [np.asarray(w, np.float32) for w in wgt]

    mlp_counts = [len(tok[e]) for e in range(NM)]
    kan_counts = [len(tok[e]) for e in range(NM, E)]
    CM = _chunk_capacity(mlp_counts, 8)
    CK = _chunk_capacity(kan_counts, 8)

    mlp_chunks = _make_chunks(
        [tok[e] for e in range(NM)], [wgt[e] for e in range(NM)], CM
    )
    kan_chunks = _make_chunks(
        [tok[e + NM] for e in range(NM)], [wgt[e + NM] for e in range(NM)], CK
    )
    n_pass = max(1, -(-len(mlp_chunks) // 8), -(-len(kan_chunks) // 8))

    # per-expert weight prep (bf16, device layout)
    kan_w = [_prep_kan_weights(sp1_w, sp2_w, e) for e in range(NM)]
    mlp_w = [_prep_mlp_weights(fc1_w, fc2_w, e) for e in range(NM)]
    fb1_h = [
        np.ascontiguousarray(np.asarray(fc1_b[e], np.float32).reshape(FTM, P).T)
        for e in range(NM)
    ]
    fb2_h = [
        np.ascontiguousarray(np.asarray(fc2_b[e], np.float32).reshape(HT, P).T)
        for e in range(NM)
    ]

    nc = _get_nc(CK, CM)

    out = np.zeros((T, H), np.float32)
    zero_k = np.zeros((H, CK), np.float32)
    zero_m = np.zeros((H, CM), np.float32)
    gb_h = np.ascontiguousarray(
        np.broadcast_to(-0.5 * _grid(), (P, G)).astype(np.float32)
    )

    for p in range(n_pass):
        in_maps = []
        metas = []  # (kan_chunk|None, mlp_chunk|None)
        for c in range(8):
            kc = kan_chunks[p * 8 + c] if p * 8 + c < len(kan_chunks) else None
            mc = mlp_chunks[p * 8 + c] if p * 8 + c < len(mlp_chunks) else None
            ke = kc[0] if kc else 0
            me = mc[0] if mc else 0
            in_maps.append(
                {
                    "xkT": _gather_tokens_T(hidden_states, kc[1], CK)
                    if kc
                    else zero_k,
                    "xmT": _gather_tokens_T(hidden_states, mc[1], CM)
                    if mc
                    else zero_m,
                    "w1": kan_w[ke][0],
                    "w2": kan_w[ke][1],
                    "fw1": mlp_w[me][0],
                    "fw2": mlp_w[me][1],
                    "fb1": fb1_h[me],
                    "fb2": fb2_h[me],
                    "gb": gb_h,
                }
            )
            metas.append((kc, mc))

        global _LAST_IN_MAPS
        _LAST_IN_MAPS = in_maps
        res = run_bass_kernel_spmd(nc, in_maps, core_ids=list(range(8)))

        for c in range(8):
            kc, mc = metas[c]
            if kc is not None and len(kc[1]):
                _, toks, w = kc
                y = res.results[c]["ykT"][:, : len(toks)]
                out[toks] += (y * w[None, :]).T
            if mc is not None and len(mc[1]):
                _, toks, w = mc
                y = res.results[c]["ymT"][:, : len(toks)]
                out[toks] += (y * w[None, :]).T

    return out


# revision 42
# speedup vs baseline: 1.0168x; 1.0168x over previous
"""MoE (4 MLP experts + 4 FasterKAN experts, top-2 routing) on 8 Trainium2
NeuronCores.

Strategy: the gate (tiny: [2048,1024]@[1024,8]) is computed on host exactly
mirroring the reference ops.  Tokens are dispatched sparsely to their selected
experts; each expert's token list is chopped into <=512-token chunks and the 8
chunks (one per core) are processed by a single SPMD Bass kernel: every core
runs one KAN expert chunk (rswaf basis -> spline matmul -> rswaf -> spline
matmul) and one MLP expert chunk (fc1 -> gelu -> fc2), bf16 matmuls with fp32
accumulation.  Host scatter-adds the weighted expert outputs.

Self-contained: hardcodes T=2048, H=1024, F=4096, E=8, G=8, top_k=2.
"""

import math

import numpy as np
import ml_dtypes

import concourse.bass as bass
import concourse.mybir as mybir
from concourse.tile import TileContext
from concourse.bass_utils import run_bass_kernel_spmd

# ---------------------------------------------------------------- constants
P = 128
T, H, F, E = 2048, 1024, 4096, 8
G = 8  # rswaf grids
TOP_K = 2
NM = E // 2  # experts per type
F2 = F // 2  # KAN hidden width

HT = H // P  # 8 h-tiles
FT1 = F2 // P  # 16 f-tiles (KAN layer 1 out)
KT1 = HT * G  # 64 k-tiles (KAN layer 1 contraction: H*G)
KT2 = FT1 * G  # 128 k-tiles (KAN layer 2 contraction: F2*G)
FTM = F // P  # 32 f-tiles (MLP hidden)

BF16 = ml_dtypes.bfloat16

_GRID = None  # lazy: match jnp.linspace bitwise


def _grid():
    global _GRID
    if _GRID is None:
        import jax.numpy as jnp

        _GRID = np.asarray(jnp.linspace(-1.2, 0.2, G).astype(jnp.float32))
    return _GRID


# ------------------------------------------------- BIR fixup for this walrus
def _split_sync_waits(nc, max_waits=1):
    """This walrus build only allows one sync-wait command per instruction;
    move extra waits onto preceding same-engine NoOps."""
    for f in nc.m.functions:
        for bb in f.blocks:
            insts = bb.instructions
            new = []
            changed = False
            for inst in insts:
                si = inst.sync_info
                if si is not None and si.on_wait and len(si.on_wait) > max_waits:
                    waits = list(si.on_wait)
                    extra = waits[:-max_waits]
                    for s in range(0, len(extra), max_waits):
                        chunk = extra[s : s + max_waits]
                        new.append(
                            mybir.InstNoOp(
                                name=nc.get_next_instruction_name(),
                                engine=inst.engine,
                                sync_info=mybir.SyncInfo(
                                    on_wait=list(chunk), on_update=[]
                                ),
                                bass_nofuse=True,
                            )
                        )
                    while len(si.on_wait) > max_waits:
                        si.on_wait.pop(0)
                    changed = True
                new.append(inst)
            if changed:
                bb.instructions = new


# ------------------------------------------------------------ device kernel
def _build_nc(CK, CM):
    """Build the per-core SPMD program (v2: KAN matmuls operand-swapped).

    KAN layers keep the token/basis tile STATIONARY in the PE array and
    stream the (host-negated) weights as the N=512 moving operand — 4x
    fewer, 2x wider PE instructions than weight-stationary form (the PE
    sequencer is the binding resource otherwise).  The basis is computed
    as tanh^2 - 1 (one ACT tanh + one ACT square + one DVE immediate
    subtract) against negated weights, which is cheaper than 1 - tanh^2.

    KAN L1 produces k1 in [token, f2] layout; a PE-transpose pass flips
    it to [f2, token] for layer 2's stationary basis tiles.
    MLP stays weight-stationary (tokens moving): N=CM is already wide and
    padding tokens to 512 would waste more than the swap saves.

    CK must be a multiple of 128; CM a multiple of 32.
    """
    assert CK % P == 0
    NBK = CK // P  # KAN token blocks
    f32 = mybir.dt.float32
    bf16 = mybir.dt.bfloat16
    TANH = mybir.ActivationFunctionType.Tanh
    GELU = mybir.ActivationFunctionType.Gelu
    IDENT = mybir.ActivationFunctionType.Identity

    nc = bass.Bass()
    xkT = nc.dram_tensor("xkT", [H, CK], f32, kind="ExternalInput")
    xmT = nc.dram_tensor("xmT", [H, CM], f32, kind="ExternalInput")
    # KAN weights, NEGATED on host.  w1[kt, i, f] ; kt = g*HT + ht
    w1 = nc.dram_tensor("w1", [KT1, P, F2], bf16, kind="ExternalInput")
    # w2[kt2, i, h] ; kt2 = g*FT1 + ft
    w2 = nc.dram_tensor("w2", [KT2, P, H], bf16, kind="ExternalInput")
    # MLP weights: fw1 [ft, i, ht, j], fw2 [ht, i, ft, j]
    fw1 = nc.dram_tensor("fw1", [FTM, P, HT, P], bf16, kind="ExternalInput")
    fw2 = nc.dram_tensor("fw2", [HT, P, FTM, P], bf16, kind="ExternalInput")
    fb1 = nc.dram_tensor("fb1", [P, FTM], f32, kind="ExternalInput")
    fb2 = nc.dram_tensor("fb2", [P, HT], f32, kind="ExternalInput")
    # per-grid tanh biases, replicated over partitions: gb[:, g] = -0.5*grid[g]
    gb = nc.dram_tensor("gb", [P, G], f32, kind="ExternalInput")
    yk = nc.dram_tensor("yk", [CK, H], f32, kind="ExternalOutput")
    ymT = nc.dram_tensor("ymT", [H, CM], f32, kind="ExternalOutput")

    xk_r = xkT.rearrange("(o p) c -> p o c", p=P)
    xm_r = xmT.rearrange("(o p) c -> p o c", p=P)
    yk_r = yk.rearrange("(o p) c -> p o c", p=P)  # [128, NBK, H]
    ym_r = ymT.rearrange("(o p) c -> p o c", p=P)

    HC = H // 512  # 2 psum-wide chunks of the KAN L2 output
    FC = F2 // 512  # 4 psum-wide chunks of the KAN L1 output

    with TileContext(nc) as tc:
        with (
            tc.tile_pool(name="persist", bufs=1) as persist,
            tc.tile_pool(name="scratch", bufs=2) as scratch,
            tc.tile_pool(name="b2pool", bufs=2) as b2pool,
            tc.tile_pool(name="wstream", bufs=2) as wstream,
            tc.tile_pool(name="fw2stream", bufs=3) as fw2stream,
            tc.tile_pool(name="w1stream", bufs=4) as w1stream,
            tc.tile_pool(name="w2stream", bufs=3) as w2stream,
            tc.tile_pool(name="outp", bufs=4) as outp,
        ):
            # ---- persistent SBUF tensors
            xk_sb = persist.tile([P, HT, CK], f32, tag="xk")
            xmb_sb = persist.tile([P, HT, CM], bf16, tag="xmb")
            bn1_sb = persist.tile([P, KT1, CK], bf16, tag="bn1")
            k1_sb = persist.tile([P, NBK, F2], f32, tag="k1")
            k1T_sb = persist.tile([P, FT1, CK], f32, tag="k1T")
            h1_sb = persist.tile([P, FTM, CM], bf16, tag="h1")
            fb1_sb = persist.tile([P, FTM], f32, tag="fb1")
            fb2_sb = persist.tile([P, HT], f32, tag="fb2")
            gb_sb = persist.tile([P, G], f32, tag="gb")
            ident = persist.tile([P, P], f32, tag="ident")

            # basis scratch width (f32 elems): b1 slabs, b2 half-slabs, xm
            BT = max(FT1 * CK // 2, HT * CK, HT * CM)

            # startup order matters: xmb + first fw1 feed the first PE work,
            # so only those two transfers go ahead of everything else.
            # gpsimd DMA casts f32->bf16 during the transfer (keeps the DVE
            # cast off the critical path; SP stays free for weight streams).
            nc.gpsimd.dma_start(xmb_sb[:], xm_r)
            nc.sync.dma_start(fb1_sb[:], fb1[:])
            from concourse.masks import make_identity

            make_identity(nc, ident[:])

            def basis_neg_wide(src_ap, g, out_ap, n):
                # out = tanh(0.5*src - 0.5*grid[g])^2 - 1 over an [P, n] slab
                # (src/out flattened to 2D by callers)
                t = scratch.tile([P, BT], f32, tag="bt", name=f"bt_g{g}_{n}")
                nc.scalar.activation(
                    t[:, :n], src_ap, TANH, bias=gb_sb[:, g : g + 1], scale=0.5
                )
                nc.scalar.square(t[:, :n], t[:, :n])
                nc.vector.tensor_scalar_sub(out_ap, t[:, :n], 1.0)

            # emission: MLP L1 (with bn1 production interleaved), MLP L2, K1,
            # transpose, K2.  Tile schedules by deps; emission order sets the
            # per-engine FIFO order.
            with (
                tc.tile_pool(name="ps_m1", bufs=3, space="PSUM") as ps_m1,
                tc.tile_pool(name="ps_m2", bufs=2, space="PSUM") as ps_m2,
            ):
                # ---------------- MLP layer 1 + bn1 basis production
                fw1_grp = None
                for ft in range(FTM):
                    if ft % 2 == 0:
                        fw1_grp = wstream.tile(
                            [P, 2, HT, P], bf16, tag="fw1", name=f"fw1g{ft}"
                        )
                        nc.sync.dma_start(
                            fw1_grp[:], fw1[ft : ft + 2].rearrange("k p o j -> p k o j")
                        )
                    fw1_sb = fw1_grp[:, ft % 2]
                    ps = ps_m1.tile([P, CM], f32, tag="m1")
                    for ht in range(HT):
                        nc.tensor.matmul(
                            ps[:],
                            fw1_sb[:, ht],
                            xmb_sb[:, ht],
                            start=(ht == 0),
                            stop=(ht == HT - 1),
                        )
                    nc.scalar.activation(
                        h1_sb[:, ft], ps[:], GELU, bias=fb1_sb[:, ft : ft + 1], scale=1.0
                    )
                    if ft == 0:
                        # non-critical loads, after the startup-critical ones
                        nc.sync.dma_start(xk_sb[:], xk_r)
                        nc.sync.dma_start(fb2_sb[:], fb2[:])
                        nc.sync.dma_start(gb_sb[:], gb[:])
                    # one per-grid bn1 slab every 4 ft iterations
                    if ft % 4 == 0 and ft > 0:
                        g = ft // 4 - 1
                        basis_neg_wide(
                            xk_sb[:].rearrange("p o c -> p (o c)"),
                            g,
                            bn1_sb[:, g * HT : (g + 1) * HT, :].rearrange(
                                "p o c -> p (o c)"
                            ),
                            HT * CK,
                        )

                # last bn1 slab (g=7) lands at the start of MLP L2
                for g in (7,):
                    basis_neg_wide(
                        xk_sb[:].rearrange("p o c -> p (o c)"),
                        g,
                        bn1_sb[:, g * HT : (g + 1) * HT, :].rearrange(
                            "p o c -> p (o c)"
                        ),
                        HT * CK,
                    )

                # ---------------- MLP layer 2
                for ht in range(HT):
                    fw2_sb = fw2stream.tile([P, FTM, P], bf16, tag="fw2")
                    nc.sync.dma_start(fw2_sb[:], fw2[ht])
                    ps = ps_m2.tile([P, CM], f32, tag="m2")
                    for ft in range(FTM):
                        nc.tensor.matmul(
                            ps[:],
                            fw2_sb[:, ft],
                            h1_sb[:, ft],
                            start=(ft == 0),
                            stop=(ft == FTM - 1),
                        )
                    out_sb = outp.tile([P, CM], f32, tag="ym")
                    nc.scalar.activation(
                        out_sb[:], ps[:], IDENT, bias=fb2_sb[:, ht : ht + 1], scale=1.0
                    )
                    nc.sync.dma_start(ym_r[:, ht], out_sb[:])

            # ---------------- KAN layer 1: stationary bn1 token-blocks,
            # moving w1 (N=512), psum [token, f2-chunk]
            w1_last_dma = None
            with tc.tile_pool(name="ps_k1", bufs=1, space="PSUM") as ps_k1:
                for bp in range(0, NBK, 2):
                    blocks = [b for b in (bp, bp + 1) if b < NBK]
                    accs = {
                        (b, fc): ps_k1.tile(
                            [P, 512], f32, tag=f"k1ps_{b - bp}_{fc}",
                            name=f"k1ps_{b}_{fc}",
                        )
                        for b in blocks
                        for fc in range(FC)
                    }
                    for kt in range(KT1):
                        w1_sb = w1stream.tile([P, F2], bf16, tag="w1")
                        w1_last_dma = nc.sync.dma_start(w1_sb[:], w1[kt]).ins
                        for b in blocks:
                            stat = bn1_sb[:, kt, b * P : (b + 1) * P]
                            for fc in range(FC):
                                nc.tensor.matmul(
                                    accs[(b, fc)][:],
                                    stat,
                                    w1_sb[:, fc * 512 : (fc + 1) * 512],
                                    start=(kt == 0),
                                    stop=(kt == KT1 - 1),
                                )
                    for fc in range(FC):
                        for b in blocks:
                            nc.vector.tensor_copy(
                                k1_sb[:, b, fc * 512 : (fc + 1) * 512],
                                accs[(b, fc)][:],
                            )

            # ---------------- transpose k1 [tok, f2] -> k1T [f2, tok]
            # fc-chunk order so K2's first basis slab unblocks early
            with tc.tile_pool(name="ps_tp", bufs=4, space="PSUM") as ps_tp:
                for fc in range(FC):
                    for ft in range(fc * (FT1 // FC), (fc + 1) * (FT1 // FC)):
                        for b in range(NBK):
                            tp = ps_tp.tile([P, P], f32, tag="tp")
                            nc.tensor.transpose(
                                tp[:], k1_sb[:, b, ft * P : (ft + 1) * P], ident[:]
                            )
                            nc.vector.tensor_copy(
                                k1T_sb[:, ft, b * P : (b + 1) * P], tp[:]
                            )

            # ---------------- KAN layer 2: stationary b2 token-blocks,
            # moving w2 (N=512), psum [token, h-chunk]; basis built per-grid
            # as one [P, FT1*CK] slab
            with tc.tile_pool(name="ps_k2", bufs=1, space="PSUM") as ps_k2:
                accs2 = {
                    (b, hc): ps_k2.tile(
                        [P, 512], f32, tag=f"acc_{b}_{hc}", name=f"acc_{b}_{hc}"
                    )
                    for b in range(NBK)
                    for hc in range(HC)
                }
                for g in range(G):
                    b2n = b2pool.tile([P, FT1, CK], bf16, tag="b2", name=f"b2n{g}")
                    # chunked basis: quarters for g=0 (unblocks K2 right after
                    # the first transposes), halves otherwise
                    n_chunks = 4 if g == 0 else 2
                    step = FT1 // n_chunks
                    w2_grp = None
                    for ci in range(n_chunks):
                        lo, hi = ci * step, (ci + 1) * step
                        basis_neg_wide(
                            k1T_sb[:, lo:hi, :].rearrange("p o c -> p (o c)"),
                            g,
                            b2n[:, lo:hi, :].rearrange("p o c -> p (o c)"),
                            step * CK,
                        )
                        for ft in range(lo, hi):
                            kt2 = g * FT1 + ft
                            if kt2 % 2 == 0:
                                w2_grp = w2stream.tile(
                                    [P, 2, H], bf16, tag="w2", name=f"w2g{kt2}"
                                )
                                w2_dma = nc.sync.dma_start(
                                    w2_grp[:],
                                    w2[kt2 : kt2 + 2].rearrange("k p h -> p k h"),
                                )
                                # keep early w2 prefetches from stealing the
                                # DMA engines out from under late-K1 w1 loads
                                if kt2 < 6 and w1_last_dma is not None:
                                    from concourse.tile import add_dep_helper

                                    add_dep_helper(
                                        w1_last_dma,
                                        w2_dma.ins,
                                        sync=False,
                                        reason="defer w2 prefetch past w1",
                                    )
                            w2_sb = w2_grp[:, kt2 % 2]
                            for b in range(NBK):
                                stat = b2n[:, ft, b * P : (b + 1) * P]
                                for hc in range(HC):
                                    nc.tensor.matmul(
                                        accs2[(b, hc)][:],
                                        stat,
                                        w2_sb[:, hc * 512 : (hc + 1) * 512],
                                        start=(kt2 == 0),
                                        stop=(kt2 == KT2 - 1),
                                    )
                for b in range(NBK):
                    for hc in range(HC):
                        out_sb = outp.tile([P, 512], f32, tag="yk")
                        nc.vector.tensor_copy(out_sb[:], accs2[(b, hc)][:])
                        nc.sync.dma_start(
                            yk_r[:, b, hc * 512 : (hc + 1) * 512], out_sb[:]
                        )

    _split_sync_waits(nc, max_waits=1)
    return nc


_NC_CACHE = {}
_LAST_IN_MAPS = None  # debug/timing hook for test.py


def _get_nc(CK, CM):
    key = (CK, CM)
    if key not in _NC_CACHE:
        _NC_CACHE[key] = _build_nc(CK, CM)
    return _NC_CACHE[key]


# ------------------------------------------------------------- host helpers
def _routing(hidden_states, gate_w):
    """Mirror the reference router bit-exactly (same ops, same order)."""
    import jax
    import jax.numpy as jnp

    logits = hidden_states @ gate_w.T
    probs = jax.nn.softmax(jnp.asarray(logits).astype(jnp.float32), axis=-1)
    rw, sel = jax.lax.top_k(probs, TOP_K)
    rw = rw / rw.sum(axis=-1, keepdims=True)
    return np.asarray(rw).astype(np.float32), np.asarray(sel)


def _chunk_capacity(counts, max_chunks, step=32):
    for C in range(step, 512 + step, step):
        if sum(-(-c // C) for c in counts if c) <= max_chunks:
            return C
    return 512  # multi-pass handles overflow


def _make_chunks(tok_lists, wgt_lists, C):
    """[(expert, token_index_array, weight_array), ...], each <= C tokens."""
    chunks = []
    for e, (toks, wgts) in enumerate(zip(tok_lists, wgt_lists)):
        for s in range(0, len(toks), C):
            chunks.append((e, toks[s : s + C], wgts[s : s + C]))
    return chunks


def _prep_kan_weights(sp1_w, sp2_w, e):
    """NEGATED device layouts (basis is computed as tanh^2 - 1 on device)."""
    # w1[kt=g*HT+ht, i, f] = -sp1_w[e][f, (ht*P+i)*G+g]
    a = np.asarray(sp1_w[e], np.float32)  # [F2, H*G]
    a = a.reshape(F2, HT, P, G).transpose(3, 1, 2, 0)  # [g, ht, i, f]
    w1 = np.ascontiguousarray(-a).reshape(KT1, P, F2).astype(BF16)
    # w2[kt2=g*FT1+ft, i, h] = -sp2_w[e][h, (ft*P+i)*G+g]
    b = np.asarray(sp2_w[e], np.float32)  # [H, F2*G]
    b = b.reshape(H, FT1, P, G).transpose(3, 1, 2, 0)  # [g, ft, i, h]
    w2 = np.ascontiguousarray(-b).reshape(KT2, P, H).astype(BF16)
    return w1, w2


def _prep_mlp_weights(fc1_w, fc2_w, e):
    fw1 = np.ascontiguousarray(
        fc1_w[e].reshape(FTM, P, HT, P).transpose(0, 3, 2, 1)
    ).astype(BF16)
    fw2 = np.ascontiguousarray(
        fc2_w[e].reshape(HT, P, FTM, P).transpose(0, 3, 2, 1)
    ).astype(BF16)
    return fw1, fw2


def _gather_tokens_T(hidden_states, toks, C):
    """[len(toks), H] tokens -> zero-padded transposed [H, C] fp32."""
    out = np.zeros((H, C), np.float32)
    if len(toks):
        out[:, : len(toks)] = hidden_states[toks].T
    return out


# ------------------------------------------------------------------- kernel
def kernel(hidden_states, gate_w, fc1_w, fc1_b, fc2_w, fc2_b, sp1_w, sp2_w):
    hidden_states = np.asarray(hidden_states, np.float32)
    gate_w = np.asarray(gate_w, np.float32)

    rw, sel = _routing(hidden_states, gate_w)

    # per-expert token lists + combine weights
    tok = [[] for _ in range(E)]
    wgt = [[] for _ in range(E)]
    for k in range(TOP_K):
        for t_idx, e in enumerate(sel[:, k]):
            tok[e].append(t_idx)
            wgt[e].append(rw[t_idx, k])
    tok = [np.asarray(t, np.int64) for t in tok]
    wgt = [np.asarray(w, np.float32) for w in wgt]

    mlp_counts = [len(tok[e]) for e in range(NM)]
    kan_counts = [len(tok[e]) for e in range(NM, E)]
    CM = _chunk_capacity(mlp_counts, 8, step=32)
    CK = _chunk_capacity(kan_counts, 8, step=128)

    mlp_chunks = _make_chunks(
        [tok[e] for e in range(NM)], [wgt[e] for e in range(NM)], CM
    )
    kan_chunks = _make_chunks(
        [tok[e + NM] for e in range(NM)], [wgt[e + NM] for e in range(NM)], CK
    )
    n_pass = max(1, -(-len(mlp_chunks) // 8), -(-len(kan_chunks) // 8))

    # per-expert weight prep (bf16, device layout)
    kan_w = [_prep_kan_weights(sp1_w, sp2_w, e) for e in range(NM)]
    mlp_w = [_prep_mlp_weights(fc1_w, fc2_w, e) for e in range(NM)]
    fb1_h = [
        np.ascontiguousarray(np.asarray(fc1_b[e], np.float32).reshape(FTM, P).T)
        for e in range(NM)
    ]
    fb2_h = [
        np.ascontiguousarray(np.asarray(fc2_b[e], np.float32).reshape(HT, P).T)
        for e in range(NM)
    ]

    nc = _get_nc(CK, CM)

    out = np.zeros((T, H), np.float32)
    zero_k = np.zeros((H, CK), np.float32)
    zero_m = np.zeros((H, CM), np.float32)
    gb_h = np.ascontiguousarray(
        np.broadcast_to(-0.5 * _grid(), (P, G)).astype(np.float32)
    )

    for p in range(n_pass):
        in_maps = []
        metas = []  # (kan_chunk|None, mlp_chunk|None)
        for c in range(8):
            kc = kan_chunks[p * 8 + c] if p * 8 + c < len(kan_chunks) else None
            mc = mlp_chunks[p * 8 + c] if p * 8 + c < len(mlp_chunks) else None
            ke = kc[0] if kc else 0
            me = mc[0] if mc else 0
            in_maps.append(
                {
                    "xkT": _gather_tokens_T(hidden_states, kc[1], CK)
                    if kc
                    else zero_k,
                    "xmT": _gather_tokens_T(hidden_states, mc[1], CM)
                    if mc
                    else zero_m,
                    "w1": kan_w[ke][0],
                    "w2": kan_w[ke][1],
                    "fw1": mlp_w[me][0],
                    "fw2": mlp_w[me][1],
                    "fb1": fb1_h[me],
                    "fb2": fb2_h[me],
                    "gb": gb_h,
                }
            )
            metas.append((kc, mc))

        global _LAST_IN_MAPS
        _LAST_IN_MAPS = in_maps
        res = run_bass_kernel_spmd(nc, in_maps, core_ids=list(range(8)))

        for c in range(8):
            kc, mc = metas[c]
            if kc is not None and len(kc[1]):
                _, toks, w = kc
                y = res.results[c]["yk"][: len(toks)]
                out[toks] += y * w[:, None]
            if mc is not None and len(mc[1]):
                _, toks, w = mc
                y = res.results[c]["ymT"][:, : len(toks)]
                out[toks] += (y * w[None, :]).T

    return out
